# revision 1
# baseline (speedup 1.0000x reference)
"""Trainium2 Bass kernel for nn_Attention_89670327206161.

Dense transformer attention block, B=8 S=4096 D=1024 H=16 (dh=64), fp32.
The reference contracts attention scores over the *sequence* axis:
    scores_h = K_h^T Q_h / sqrt(dh)   -> (dh, dh) per head
    P_h      = softmax(scores_h, axis=-1)
    out_h    = V_h @ P_h              -> (S, dh)
    out      = concat_h(out_h) @ Wo^T

Sharding: pure data parallelism over batch -- one batch element per
NeuronCore, no collectives. Per core everything streams through SBUF;
only x (transposed on host), the four transposed weights, and the output
touch HBM.

Matmuls run in float32r (full-rate fp32 on the PE at N>=256).

Phase A: stream 32 seq-tiles of 128; compute Q,K (seq-major) with
  PSUM-accumulated projections, immediately accumulate head-pair score
  blocks (two heads packed -> 128x128) into a persistent PSUM tile.
Softmax: max-subtracted exp(0.125*(x - max)) per head block (logits
  reach |142| on real data, so max-subtraction is mandatory),
  row-normalized into a block-diagonal P tile per pair.
Phase B: stream 8 seq-blocks of 512; compute V^T (feature-major),
  attention out Z^T = blockdiag(P)^T @ V^T per pair, then the output
  projection back to seq-major, DMA to HBM.
"""

import numpy as np

HEADS = 16
B, S, D = 8, 4096, 1024
DH = D // HEADS          # 64
NPAIR = HEADS // 2       # 8 head pairs -> 128-wide blocks
P = 128                  # partitions
NKC = D // P             # 8 contraction chunks of 128
NT_A = S // P            # 32 seq tiles in phase A
S_BLK = 512
NT_B = S // S_BLK        # 8 seq blocks in phase B
N_CORES = 8

_PROGRAM = None


def _ts(i, n):
    return slice(i * n, (i + 1) * n)


def _build_program():
    import concourse.bacc as bacc
    import concourse.mybir as mybir
    import concourse.tile as tile

    f32 = mybir.dt.float32
    f32r = mybir.dt.float32r
    EXP = mybir.ActivationFunctionType.Exp
    X = mybir.AxisListType.X

    nc = bacc.Bacc(trn_type="TRN2", debug=False, num_devices=N_CORES)

    xT = nc.dram_tensor("xT", [D, S], f32r, kind="ExternalInput")
    wqT = nc.dram_tensor("wqT", [D, D], f32r, kind="ExternalInput")
    wkT = nc.dram_tensor("wkT", [D, D], f32r, kind="ExternalInput")
    wvT = nc.dram_tensor("wvT", [D, D], f32r, kind="ExternalInput")
    woT = nc.dram_tensor("woT", [D, D], f32r, kind="ExternalInput")
    out = nc.dram_tensor("out", [S, D], f32, kind="ExternalOutput")

    xTr = xT.ap().rearrange("(c p) s -> p c s", p=P)      # (128, 8, 4096)

    with tile.TileContext(nc) as tc:
      with (
          tc.tile_pool(name="persist", bufs=1) as persist_pool,
          # scores PSUM stays reserved through phase B so b_ps lands in the
          # released qk banks instead -- lets the first V^T matmuls overlap
          # the softmax (which is still reading the score banks)
          tc.tile_pool(name="sc_ps", bufs=1, space="PSUM") as sc_ps_pool,
          tc.tile_pool(name="xb", bufs=2) as xb_pool,
      ):
        p_all_pool = persist_pool
        wv_sb = persist_pool.tile([P, NKC, D], f32r, tag="wv")
        wo_sb = persist_pool.tile([P, NKC, D], f32r, tag="wo")
        with (
            tc.tile_pool(name="w_qk", bufs=1) as w_qk_pool,
            tc.tile_pool(name="const", bufs=1) as const_pool,
        ):
            wq_sb = w_qk_pool.tile([P, NKC, D], f32r, tag="wq")
            wk_sb = w_qk_pool.tile([P, NKC, D], f32r, tag="wk")
            # phase-A-critical weights first in the DMA queues; V/O weights
            # queued after (they are only needed in phase B)
            # issue in PE consumption order (Q oh0, Q oh1, K oh0, K oh1) so
            # the first s-tile's matmul groups unblock progressively
            wqTr = wqT.ap().rearrange("(c p) o -> p c o", p=P)
            wkTr = wkT.ap().rearrange("(c p) o -> p c o", p=P)
            nc.sync.dma_start(wq_sb[:, :, 0:512], wqTr[:, :, 0:512])

            zero_sb = const_pool.tile([P, 512], f32r)
            nc.vector.memset(zero_sb[:].bitcast(f32), 0.0)

            # All 8 head-pair score blocks live in one 4-bank PSUM tile for
            # the whole of phase A. Each pair's matmul uses a 256-wide rhs
            # (two pairs of Q columns) so float32r streams at full rate
            # (N>=256); the non-matching 128-col half of each output block is
            # garbage that is never read. Dummy start=True matmuls clear the
            # has_written bits bank-wide; every real score matmul then
            # accumulates with start=False (order-independent).
            scores_ps = sc_ps_pool.tile([P, NPAIR * 256], f32)
            # HAM warm-up: ~5us of zero matmuls (gated only on the memset)
            # spin the PE to K=8/8 while the first weight DMAs are in
            # flight; each is an idempotent extra clear of a score bank.
            for w in range(3):
                for bank in range(4):
                    nc.tensor.matmul(
                        scores_ps[:, bank * 512:bank * 512 + P],
                        zero_sb[:, 0:P], zero_sb[:, 0:P],
                        start=True, stop=False, skip_group_check=True,
                    )
            for bank in range(4):
                nc.tensor.matmul(
                    scores_ps[:, _ts(bank, 512)],
                    zero_sb[:, 0:P],
                    zero_sb[:],
                    start=True, stop=False, skip_group_check=True,
                )

            with (
                tc.tile_pool(name="xa", bufs=3) as xa_pool,
                tc.tile_pool(name="qk_sb", bufs=2) as qk_sb_pool,
                tc.tile_pool(name="qk_ps", bufs=4, space="PSUM") as qk_ps_pool,
            ):
                # prefetch the first two x slabs and the remaining weight
                # halves in the order PE will consume them; phase-B weights
                # (wv/wo) queue last
                xa_pre = []
                xa = xa_pool.tile([P, NKC, P], f32r, tag="xa")
                nc.sync.dma_start(xa[:], xTr[:, :, _ts(0, P)])
                xa_pre.append(xa)
                nc.sync.dma_start(wk_sb[:, :, 0:512], wkTr[:, :, 0:512])
                nc.sync.dma_start(wq_sb[:, :, 512:D], wqTr[:, :, 512:D])
                nc.sync.dma_start(wk_sb[:, :, 512:D], wkTr[:, :, 512:D])
                xa = xa_pool.tile([P, NKC, P], f32r, tag="xa")
                nc.sync.dma_start(xa[:], xTr[:, :, _ts(1, P)])
                xa_pre.append(xa)
                # background transfers go through SWDGE so they don't sit
                # ahead of the critical xa stream in the HWDGE FIFOs
                nc.gpsimd.dma_start(wv_sb[:], wvT.ap().rearrange("(c p) o -> p c o", p=P))
                nc.gpsimd.dma_start(wo_sb[:], woT.ap().rearrange("(c p) o -> p c o", p=P))
                # prefetch the first phase-B x slab so V^T starts immediately
                xb_pre = xb_pool.tile([P, NKC, S_BLK], f32r, tag="xb")
                nc.gpsimd.dma_start(xb_pre[:], xTr[:, :, _ts(0, S_BLK)])

                for it in range(NT_A):
                    if it < 2:
                        xa = xa_pre[it]
                    else:
                        xa = xa_pool.tile([P, NKC, P], f32r, tag="xa")
                        nc.sync.dma_start(xa[:], xTr[:, :, _ts(it, P)])

                    q_sb = qk_sb_pool.tile([P, D], f32r, tag="q")
                    k_sb = qk_sb_pool.tile([P, D], f32r, tag="k")
                    if it == 0:
                        # match the weight-DMA arrival order during the ramp
                        groups = [(wq_sb, q_sb, True, 0), (wk_sb, k_sb, False, 0),
                                  (wq_sb, q_sb, True, 1), (wk_sb, k_sb, False, 1)]
                    else:
                        groups = [(wq_sb, q_sb, True, 0), (wq_sb, q_sb, True, 1),
                                  (wk_sb, k_sb, False, 0), (wk_sb, k_sb, False, 1)]
                    for w_sb, dst, on_act, oh in groups:
                        ps = qk_ps_pool.tile([P, 512], f32, tag="qkps")
                        for ic in range(NKC):
                            nc.tensor.matmul(
                                ps[:], xa[:, ic, :], w_sb[:, ic, _ts(oh, 512)],
                                start=(ic == 0), stop=(ic == NKC - 1),
                            )
                        if on_act:
                            nc.scalar.copy(dst[:, _ts(oh, 512)], ps[:])
                        else:
                            nc.vector.tensor_copy(dst[:, _ts(oh, 512)], ps[:])

                    for pr in range(NPAIR):
                        nc.tensor.matmul(
                            scores_ps[:, _ts(pr, 256)],
                            k_sb[:, _ts(pr, P)], q_sb[:, _ts(pr // 2, 256)],
                            start=False, stop=False, skip_group_check=True,
                        )

            # ---- softmax over the q axis of each (dh x dh) head block ----
            p_all = p_all_pool.tile([P, NPAIR, P], f32r)
            nc.vector.memset(p_all[:].bitcast(f32), 0.0)
            with tc.tile_pool(name="smx", bufs=4) as smx_pool:
                for pr in range(NPAIR):
                    # wanted half of the 256-wide output block for pair pr
                    base = pr * 256 + (pr % 2) * P
                    for hf in range(2):
                        rows = slice(64 * hf, 64 * hf + 64)
                        cols = slice(base + 64 * hf, base + 64 * hf + 64)
                        pcols = slice(64 * hf, 64 * hf + 64)
                        # logits reach |142| on real data -- max-subtraction
                        # is required to keep exp inside fp32 range
                        mx = smx_pool.tile([P, 1], f32, tag="mx")
                        nmx = smx_pool.tile([P, 1], f32, tag="nmx")
                        nc.vector.reduce_max(
                            mx[rows, 0:1], scores_ps[rows, cols], axis=X, negate=True
                        )
                        nc.vector.tensor_scalar_mul(nmx[rows, 0:1], mx[rows, 0:1], 0.125)
                        p_tmp = smx_pool.tile([P, 64], f32, tag="ptmp")
                        nc.scalar.activation(
                            p_tmp[rows, :], scores_ps[rows, cols], EXP,
                            bias=nmx[rows, 0:1], scale=0.125,
                        )
                        den = smx_pool.tile([P, 1], f32, tag="den")
                        rec = smx_pool.tile([P, 1], f32, tag="rec")
                        nc.vector.reduce_sum(den[rows, 0:1], p_tmp[rows, :], axis=X)
                        nc.vector.reciprocal(rec[rows, 0:1], den[rows, 0:1])
                        nc.vector.tensor_scalar_mul(
                            p_all[rows, pr, pcols], p_tmp[rows, :], rec[rows, 0:1]
                        )

        # ---- phase B: V^T, attention out, output projection ----
        with (
            tc.tile_pool(name="vt", bufs=2) as vt_pool,
            tc.tile_pool(name="zt", bufs=2) as zt_pool,
            tc.tile_pool(name="ob", bufs=2) as ob_pool,
            tc.tile_pool(name="b_ps", bufs=4, space="PSUM") as b_ps_pool,
        ):
            def emit_vt(ib):
                if ib == 0:
                    xb = xb_pre
                else:
                    xb = xb_pool.tile([P, NKC, S_BLK], f32r, tag="xb")
                    nc.sync.dma_start(xb[:], xTr[:, :, _ts(ib, S_BLK)])
                vt_sb = vt_pool.tile([P, NKC, S_BLK], f32r, tag="vt")
                for oc in range(NKC):
                    ps = b_ps_pool.tile([P, S_BLK], f32, tag="bps")
                    for ic in range(NKC):
                        nc.tensor.matmul(
                            ps[:], wv_sb[:, ic, _ts(oc, P)], xb[:, ic, :],
                            start=(ic == 0), stop=(ic == NKC - 1),
                        )
                    if oc % 2 == 0:
                        nc.scalar.copy(vt_sb[:, oc, :], ps[:])
                    else:
                        nc.vector.tensor_copy(vt_sb[:, oc, :], ps[:])
                return vt_sb

            # software-pipeline V^T one block ahead: VT(0) and VT(1) are
            # both emitted before the first attention matmul, doubling the
            # PE work available to hide the softmax at the transition
            vt_tiles = [emit_vt(0), emit_vt(1)]
            for ib in range(NT_B):
                vt_sb = vt_tiles[ib]

                zt_sb = zt_pool.tile([P, NKC, S_BLK], f32r, tag="zt")
                for pr in range(NPAIR):
                    ps = b_ps_pool.tile([P, S_BLK], f32, tag="bps")
                    nc.tensor.matmul(
                        ps[:], p_all[:, pr, :], vt_sb[:, pr, :],
                        start=True, stop=True,
                    )
                    if pr % 2 == 0:
                        nc.vector.tensor_copy(zt_sb[:, pr, :], ps[:])
                    else:
                        nc.scalar.copy(zt_sb[:, pr, :], ps[:])

                for st in range(S_BLK // P):
                    o_sb = ob_pool.tile([P, D], f32, tag="ob")
                    for ot in range(2):
                        ps = b_ps_pool.tile([P, 512], f32, tag="bps")
                        for jc in range(NKC):
                            nc.tensor.matmul(
                                ps[:], zt_sb[:, jc, _ts(st, P)],
                                wo_sb[:, jc, _ts(ot, 512)],
                                start=(jc == 0), stop=(jc == NKC - 1),
                            )
                        if ot == 0:
                            nc.scalar.copy(o_sb[:, _ts(ot, 512)], ps[:])
                        else:
                            nc.vector.tensor_copy(o_sb[:, _ts(ot, 512)], ps[:])
                    r0 = ib * S_BLK + st * P
                    nc.sync.dma_start(out.ap()[r0:r0 + P, :], o_sb[:])
                if ib + 2 < NT_B:
                    vt_tiles.append(emit_vt(ib + 2))

    nc.compile()
    return nc


def _get_program():
    global _PROGRAM
    if _PROGRAM is None:
        _PROGRAM = _build_program()
    return _PROGRAM


def kernel(x, Wq, Wk, Wv, Wo):
    from concourse import bass_utils

    nc = _get_program()

    xT_all = np.ascontiguousarray(np.transpose(np.asarray(x, np.float32), (0, 2, 1)))
    wqT = np.ascontiguousarray(np.asarray(Wq, np.float32).T)
    wkT = np.ascontiguousarray(np.asarray(Wk, np.float32).T)
    wvT = np.ascontiguousarray(np.asarray(Wv, np.float32).T)
    woT = np.ascontiguousarray(np.asarray(Wo, np.float32).T)

    in_maps = [
        {"xT": xT_all[b], "wqT": wqT, "wkT": wkT, "wvT": wvT, "woT": woT}
        for b in range(N_CORES)
    ]
    res = bass_utils.run_bass_kernel_spmd(nc, in_maps, core_ids=list(range(N_CORES)))
    return np.stack([res.results[b]["out"] for b in range(N_CORES)], axis=0)



# revision 6
# speedup vs baseline: 2.8899x; 2.8899x over previous
"""Trainium2 Bass kernel for nn_Attention_89670327206161.

Dense transformer attention block, B=8 S=4096 D=1024 H=16 (dh=64), fp32.
The reference contracts attention scores over the *sequence* axis:
    scores_h = K_h^T Q_h / sqrt(dh)   -> (dh, dh) per head
    P_h      = softmax(scores_h, axis=-1)
    out_h    = V_h @ P_h              -> (S, dh)
    out      = concat_h(out_h) @ Wo^T

Because P_h is position-independent, the whole pipeline collapses
algebraically (exactly, no approximation):
    G        = x^T x                      (1024x1024 Gram, symmetric)
    scores_h = Wk_h G Wq_h^T              (== K_h^T Q_h)
    M        = Wv^T blockdiag(P_h) Wo^T   (1024x1024)
    out      = x @ M

This does ~10.9e9 MACs/core instead of ~17.8e9 for the direct
projection route (Q/K/V/O GEMMs): G (symmetric-half) + A = G Wq^T +
pair-packed Wk reduction + small M build + one output GEMM.

Sharding: pure data parallelism over batch -- one batch element per
NeuronCore, no collectives.

dtypes: score path (x_seq, G, Wq, Wk, A, softmax) is fp32/f32r --
logits reach |142| so they need ~1e-4 relative accuracy.  The output
path (P, Wv, Wo^T, DS, M, x^T) is bf16: its ~0.3% relative error is
40x under the 2e-2 gate and halves DMA+SBUF there.

Phases (per core):
  1. G = x^T x: stream 32 seq-chunks of 128 in 4 superchunks; PSUM
     accumulates upper-triangular row-block strips (pass B cols
     512:1024 for rc 0..7, pass A cols 0:512 for rc 0..3), DVE adds
     into SBUF G; 22 lower blocks mirrored via PE transpose.
  2. A = G @ Wq^T chunkwise (PSUM->SBUF), each chunk immediately
     reduced into persistent pair-packed score PSUM via Wk^T.
  3. Per-head softmax (max-subtracted exp, row-normalized) -> block-
     diagonal P pairs (bf16).
  4. DS = blockdiag(P)^T-applied Wv rows; M = DS^T-reduce with Wo^T,
     cast bf16.
  5. out = x @ M: stream x^T bf16 in 8 seq-blocks, 32 output tiles,
     DMA to HBM.
"""

import numpy as np

HEADS = 16
B, S, D = 8, 4096, 1024
P = 128                  # partitions
NKC = D // P             # 8 feature chunks of 128
NSC = S // P             # 32 seq chunks of 128
SUPER = 8                # seq chunks per superchunk
NSUP = NSC // SUPER      # 4
NPAIR = HEADS // 2       # 8 head pairs -> 128-wide blocks
N_CORES = 8

# G row-block strips: (rc, c0, c1).  Pass B covers cols 512:1024,
# pass A cols 0:512.  rc3/rc7 take full 512-wide strips (same PE cost
# as the 128-wide remnant at the <256 fp32r penalty) so their lower
# blocks come out directly and need no mirror.
G_PASS_B = [(0, 512, 1024), (1, 512, 1024), (2, 512, 1024), (3, 512, 1024),
            (4, 512, 1024), (5, 640, 1024), (6, 768, 1024), (7, 512, 1024)]
G_PASS_A = [(0, 0, 512), (1, 128, 512), (2, 256, 512), (3, 0, 512)]
# lower-triangle blocks (r, c) still needing a transpose-mirror
G_MIRRORS = [(r, c) for r in range(NKC) for c in range(r)
             if not (r == 3 and c < 3) and not (r == 7 and 4 <= c < 7)]

_PROGRAM = None


def _ts(i, n):
    return slice(i * n, (i + 1) * n)


def _build_program(repeat=1):
    # repeat>1 unrolls the whole computation R times in one program --
    # only used by measurement scripts to amplify device time above the
    # axon RPC dispatch noise.  kernel() always uses repeat=1.
    import concourse.bacc as bacc
    import concourse.mybir as mybir
    import concourse.tile as tile

    f32 = mybir.dt.float32
    f32r = mybir.dt.float32r
    bf16 = mybir.dt.bfloat16
    EXP = mybir.ActivationFunctionType.Exp
    X = mybir.AxisListType.X

    nc = bacc.Bacc(trn_type="TRN2", debug=False, num_devices=N_CORES)

    xs_d = nc.dram_tensor("xs", [S, D], f32r, kind="ExternalInput")
    xT_d = nc.dram_tensor("xTb", [D, S], bf16, kind="ExternalInput")
    wqT_d = nc.dram_tensor("wqT", [D, D], f32r, kind="ExternalInput")
    wkT_d = nc.dram_tensor("wkT", [D, D], f32r, kind="ExternalInput")
    wv_d = nc.dram_tensor("wv", [D, D], bf16, kind="ExternalInput")
    woT_d = nc.dram_tensor("woT", [D, D], bf16, kind="ExternalInput")
    eye_d = nc.dram_tensor("eye", [P, P], f32r, kind="ExternalInput")
    out_d = nc.dram_tensor("out", [S, D], f32, kind="ExternalOutput")

    xs_ap = xs_d.ap()                                        # (4096, 1024)
    xTr = xT_d.ap().rearrange("(c p) s -> p c s", p=P)       # (128, 8, 4096)
    wqTr = wqT_d.ap().rearrange("(c p) m -> p c m", p=P)
    wkTr = wkT_d.ap().rearrange("(c p) m -> p c m", p=P)
    wvr = wv_d.ap().rearrange("(c p) d -> p c d", p=P)
    woTr = woT_d.ap().rearrange("(c p) j -> p c j", p=P)

    with tile.TileContext(nc) as tc:
     for _rep in range(repeat):
      with tc.tile_pool(name="L0", bufs=1) as L0:
        zero_sb = L0.tile([P, 512], f32r, tag="zero")
        eye_sb = L0.tile([P, P], f32r, tag="eye")
        m_sb = L0.tile([P, NKC, D], bf16, tag="m")
        nc.vector.memset(zero_sb[:].bitcast(f32), 0.0)
        nc.sync.dma_start(eye_sb[:], eye_d.ap())

        with tc.tile_pool(name="Lg", bufs=1) as Lg:
          g_sb = Lg.tile([P, NKC, D], f32r, tag="g")
          with tc.tile_pool(name="Lwq", bufs=1) as Lwq:
            wq_sb = Lwq.tile([P, NKC, D], f32r, tag="wq")
            nc.gpsimd.dma_start(wq_sb[:], wqTr)

            # HAM warm-up: spin the PE on zero matmuls (gated only on
            # the memset) while the first x chunks are still in flight.
            with tc.tile_pool(name="scr_ps", bufs=1, space="PSUM") as scr:
                w_ps = scr.tile([P, 512], f32, tag="w")
                for _ in range(10):
                    nc.tensor.matmul(
                        w_ps[:], zero_sb[:, 0:P], zero_sb[:],
                        start=True, stop=False, skip_group_check=True,
                    )

            # ---- phase 1: G = x^T x (upper triangle) ----
            with (
                tc.tile_pool(name="Lxs", bufs=2 * SUPER) as Lxs,
                tc.tile_pool(name="g_ps", bufs=1, space="PSUM") as gps,
            ):
                def fetch_super(sp):
                    ts = []
                    for i in range(SUPER):
                        t = Lxs.tile([P, D], f32r, tag="xs")
                        nc.sync.dma_start(
                            t[:], xs_ap[_ts(sp * SUPER + i, P), :])
                        ts.append(t)
                    return ts

                # Bank (tag) plan: pass B tags = (rc+4)%8, pass A tags
                # = rc.  Within each pass, matmuls are emitted for the
                # longest-freed banks first and evictions run in the
                # order the *next* pass needs its banks back, so the PE
                # never waits more than ~0.3us on a DVE eviction.
                xs_tiles = fetch_super(0)
                for sp in range(NSUP):
                    nxt = fetch_super(sp + 1) if sp + 1 < NSUP else None
                    for strips, tag_of, emit_order, evict_order in (
                        (G_PASS_B, lambda rc: (rc + 4) % 8,
                         (0, 1, 2, 3, 4, 5, 6, 7), (4, 5, 6, 7, 0, 1, 2, 3)),
                        (G_PASS_A, lambda rc: rc,
                         (0, 1, 2, 3), (0, 1, 2, 3)),
                    ):
                        by_rc = {rc: (c0, c1) for rc, c0, c1 in strips}
                        ps = {rc: gps.tile([P, 512], f32, tag=f"g{tag_of(rc)}",
                                           name=f"gps{tag_of(rc)}")
                              for rc in emit_order}
                        for i, xt in enumerate(xs_tiles):
                            for rc in emit_order:
                                c0, c1 = by_rc[rc]
                                nc.tensor.matmul(
                                    ps[rc][:, 0:c1 - c0],
                                    xt[:, _ts(rc, P)], xt[:, c0:c1],
                                    start=(i == 0), stop=(i == SUPER - 1),
                                )
                        for rc in evict_order:
                            c0, c1 = by_rc[rc]
                            if sp == 0:
                                nc.vector.tensor_copy(
                                    g_sb[:, rc, c0:c1], ps[rc][:, 0:c1 - c0])
                            else:
                                nc.vector.tensor_add(
                                    g_sb[:, rc, c0:c1],
                                    g_sb[:, rc, c0:c1],
                                    ps[rc][:, 0:c1 - c0])
                    if nxt is not None:
                        xs_tiles = nxt

            # mirror the remaining lower-triangle blocks
            with tc.tile_pool(name="mir_ps", bufs=4, space="PSUM") as mps:
                for r, c in G_MIRRORS:
                    mt = mps.tile([P, P], f32r, tag="mir")
                    nc.tensor.transpose(
                        mt[:], g_sb[:, c, _ts(r, P)], eye_sb[:])
                    nc.vector.tensor_copy(g_sb[:, r, _ts(c, P)], mt[:])

            # ---- phases 2-4 ----
            with tc.tile_pool(name="Lxt", bufs=3) as Lxt:
              with tc.tile_pool(name="L3", bufs=1) as L3:
                wk_sb = L3.tile([P, NKC, D], f32r, tag="wk")
                wv_sb = L3.tile([P, NKC, D], bf16, tag="wv")
                wo_sb = L3.tile([P, NKC, D], bf16, tag="wo")
                p_all = L3.tile([P, NPAIR, P], bf16, tag="p")
                nc.gpsimd.dma_start(wk_sb[:], wkTr)
                nc.gpsimd.dma_start(wv_sb[:], wvr)
                nc.gpsimd.dma_start(wo_sb[:], woTr)
                nc.vector.memset(p_all[:], 0.0)

                with tc.tile_pool(name="sc_ps", bufs=1, space="PSUM") as scps:
                    scores_ps = scps.tile([P, NPAIR * 256], f32, tag="sc")
                    for i in range(4):
                        nc.tensor.matmul(
                            scores_ps[:, _ts(i, 512)],
                            zero_sb[:, 0:P], zero_sb[:],
                            start=True, stop=False, skip_group_check=True,
                        )

                    # A = G @ Wq^T chunkwise; each chunk feeds the
                    # pair-packed score reduction.  Software-pipelined
                    # one chunk ahead so score matmuls never wait on a
                    # fresh eviction.
                    with (
                        tc.tile_pool(name="Lab", bufs=2) as Lab,
                        tc.tile_pool(name="a_ps", bufs=2, space="PSUM") as aps,
                    ):
                        def emit_a(dc):
                            a_ps = aps.tile([P, D], f32, tag="aps")
                            for jc in range(NKC):
                                for h in range(2):
                                    nc.tensor.matmul(
                                        a_ps[:, _ts(h, 512)],
                                        g_sb[:, jc, _ts(dc, P)],
                                        wq_sb[:, jc, _ts(h, 512)],
                                        start=(jc == 0), stop=(jc == NKC - 1),
                                    )
                            a_sb = Lab.tile([P, D], f32r, tag="ab")
                            nc.scalar.copy(a_sb[:, 0:512], a_ps[:, 0:512])
                            nc.vector.tensor_copy(a_sb[:, 512:D], a_ps[:, 512:D])
                            return a_sb

                        def emit_scores(dc, a_sb):
                            for pr in range(NPAIR):
                                nc.tensor.matmul(
                                    scores_ps[:, _ts(pr, 256)],
                                    wk_sb[:, dc, _ts(pr, P)],
                                    a_sb[:, _ts(pr // 2, 256)],
                                    start=False, stop=False,
                                    skip_group_check=True,
                                )

                        prev = (0, emit_a(0))
                        for dc in range(1, NKC):
                            a_sb = emit_a(dc)
                            emit_scores(*prev)
                            prev = (dc, a_sb)
                        emit_scores(*prev)

                    # ---- softmax over q within each (64x64) head block ----
                    with tc.tile_pool(name="smx", bufs=4) as smx:
                        for pr in range(NPAIR):
                            base = pr * 256 + (pr % 2) * P
                            for hf in range(2):
                                rows = slice(64 * hf, 64 * hf + 64)
                                cols = slice(base + 64 * hf, base + 64 * hf + 64)
                                pcols = slice(64 * hf, 64 * hf + 64)
                                mx = smx.tile([P, 1], f32, tag="mx")
                                nmx = smx.tile([P, 1], f32, tag="nmx")
                                nc.vector.reduce_max(
                                    mx[rows, 0:1], scores_ps[rows, cols],
                                    axis=X, negate=True)
                                nc.vector.tensor_scalar_mul(
                                    nmx[rows, 0:1], mx[rows, 0:1], 0.125)
                                p_tmp = smx.tile([P, 64], f32, tag="pt")
                                nc.scalar.activation(
                                    p_tmp[rows, :], scores_ps[rows, cols], EXP,
                                    bias=nmx[rows, 0:1], scale=0.125)
                                den = smx.tile([P, 1], f32, tag="den")
                                rec = smx.tile([P, 1], f32, tag="rec")
                                nc.vector.reduce_sum(
                                    den[rows, 0:1], p_tmp[rows, :], axis=X)
                                nc.vector.reciprocal(
                                    rec[rows, 0:1], den[rows, 0:1])
                                nc.vector.tensor_scalar_mul(
                                    p_all[rows, pr, pcols], p_tmp[rows, :],
                                    rec[rows, 0:1])

                # ---- phase 4: DS = blockdiag(P)^T Wv ; M = DS^T Wo^T ----
                with (
                    tc.tile_pool(name="Lds", bufs=1) as Lds,
                    tc.tile_pool(name="ds_ps", bufs=2, space="PSUM") as dsps,
                    tc.tile_pool(name="m_ps", bufs=2, space="PSUM") as mmps,
                ):
                    ds_sb = Lds.tile([P, NPAIR, D], bf16, tag="ds")
                    for pr in range(NPAIR):
                        d_ps = dsps.tile([P, D], f32, tag="dsps")
                        for h in range(2):
                            nc.tensor.matmul(
                                d_ps[:, _ts(h, 512)],
                                p_all[:, pr, :], wv_sb[:, pr, _ts(h, 512)],
                                start=True, stop=True,
                            )
                        nc.scalar.copy(ds_sb[:, pr, 0:512], d_ps[:, 0:512])
                        nc.vector.tensor_copy(ds_sb[:, pr, 512:D], d_ps[:, 512:D])

                    # prefetch the x^T stream for phase 5 (WAR-gated on
                    # the released xs region, so these overlap phase 4)
                    xt_tiles = []
                    for b in range(NSC // 4):
                        t = Lxt.tile([P, NKC, 512], bf16, tag="xt")
                        nc.gpsimd.dma_start(t[:], xTr[:, :, _ts(b, 512)])
                        xt_tiles.append(t)

                    for dj in range(NKC):
                        m_ps = mmps.tile([P, D], f32, tag="mps")
                        for pr in range(NPAIR):
                            for h in range(2):
                                nc.tensor.matmul(
                                    m_ps[:, _ts(h, 512)],
                                    ds_sb[:, pr, _ts(dj, P)],
                                    wo_sb[:, pr, _ts(h, 512)],
                                    start=(pr == 0), stop=(pr == NPAIR - 1),
                                )
                        nc.scalar.copy(m_sb[:, dj, 0:512], m_ps[:, 0:512])
                        nc.vector.tensor_copy(m_sb[:, dj, 512:D], m_ps[:, 512:D])

              # ---- phase 5: out = x @ M ----
              with (
                  tc.tile_pool(name="Lob", bufs=2) as Lob,
                  tc.tile_pool(name="o_ps", bufs=3, space="PSUM") as ops,
              ):
                  for st in range(NSC):
                      xt = xt_tiles[st // 4]
                      so = st % 4
                      o_ps = ops.tile([P, D], f32, tag="ops")
                      for dc in range(NKC):
                          for h in range(2):
                              nc.tensor.matmul(
                                  o_ps[:, _ts(h, 512)],
                                  xt[:, dc, _ts(so, P)],
                                  m_sb[:, dc, _ts(h, 512)],
                                  start=(dc == 0), stop=(dc == NKC - 1),
                              )
                      o_sb = Lob.tile([P, D], f32, tag="ob")
                      nc.scalar.copy(o_sb[:, 0:512], o_ps[:, 0:512])
                      nc.vector.tensor_copy(o_sb[:, 512:D], o_ps[:, 512:D])
                      nc.sync.dma_start(out_d.ap()[_ts(st, P), :], o_sb[:])

    nc.compile()
    return nc


def _get_program():
    global _PROGRAM
    if _PROGRAM is None:
        _PROGRAM = _build_program()
    return _PROGRAM


def _prep_in_maps(x, Wq, Wk, Wv, Wo):
    import ml_dtypes

    bf = ml_dtypes.bfloat16
    x_np = np.asarray(x, np.float32)
    wqT = np.ascontiguousarray(np.asarray(Wq, np.float32).T)
    wkT = np.ascontiguousarray(np.asarray(Wk, np.float32).T)
    wv = np.ascontiguousarray(np.asarray(Wv, np.float32)).astype(bf)
    woT = np.ascontiguousarray(np.asarray(Wo, np.float32).T).astype(bf)
    eye = np.eye(P, dtype=np.float32)
    in_maps = []
    for b in range(N_CORES):
        xs = np.ascontiguousarray(x_np[b])
        xTb = np.ascontiguousarray(x_np[b].T).astype(bf)
        in_maps.append({"xs": xs, "xTb": xTb, "wqT": wqT, "wkT": wkT,
                        "wv": wv, "woT": woT, "eye": eye})
    return in_maps


def kernel(x, Wq, Wk, Wv, Wo):
    from concourse import bass_utils

    nc = _get_program()
    in_maps = _prep_in_maps(x, Wq, Wk, Wv, Wo)
    res = bass_utils.run_bass_kernel_spmd(nc, in_maps, core_ids=list(range(N_CORES)))
    return np.stack([res.results[b]["out"] for b in range(N_CORES)], axis=0)
